# revision 1
# baseline (speedup 1.0000x reference)
"""Trainium2 Bass kernel for nn_DecoderModel (2x GRU(488) + custom GRU cell(35), T=120).

Data-parallel: batch 512 sharded 64/core across 8 cores. All weights SBUF-resident.
Per-step matmuls in batch-major layout (stationary = transposed state, moving =
host-transposed weights) in float32r (single-pass PE matmul, 4x fp32 throughput
at N>=256); gate biases folded into a ones-row of the stationary ("augmented"
K=489 contraction); the linear+BN prefix and GRU0's x-side gates are host-folded
into a single [197, 1464] matmul done once in the prologue.

Softmax exp is computed as sigmoid(x-m)/sigmoid(m-x) to keep the ACT engine on a
single activation-table set (sigmoid_and_others: Sigmoid/Tanh/Copy/Identity).
"""

import os
import sys

sys.path.insert(0, "/opt/trn_rl_repo")

import numpy as np

B, HID, REC, NCH, T_FULL = 512, 196, 488, 35, 120
NCORES = 8
BC = B // NCORES          # 64 batch per core
G3 = 3 * REC              # 1464
KC = 122                  # K chunk of the 488-dim contraction
BN_EPS = 1e-3

# psum bank slicing of the 1464 gate columns: (bank, col0, width)
NSL = [(0, 0, 512), (1, 512, 464), (2, 976, 488)]


def _prep_weights(inp):
    """Host-side folding. Returns dict of np arrays shared by all cores."""
    f8 = np.float64
    W0 = inp["W0"].astype(f8); b0 = inp["b0"].astype(f8)
    g0 = inp["g0"].astype(f8).ravel(); beta0 = inp["beta0"].astype(f8).ravel()
    W1 = inp["W1"].astype(f8); b1 = inp["b1"].astype(f8)
    g1 = inp["g1"].astype(f8).ravel(); beta1 = inp["beta1"].astype(f8).ravel()
    s = 1.0 / np.sqrt(1.0 + BN_EPS)

    W0e = W0.T * (g0 * s)[None, :]
    b0e = b0 * (g0 * s) + beta0
    W1e = W1.T * (g1 * s)[None, :]
    b1e = b1 * (g1 * s) + beta1
    Wz = W0e @ W1e                      # [196,196]
    bz = b0e @ W1e + b1e                # [196]

    Wih0 = inp["gru0_Wih"].astype(f8)   # [1464, 196]
    bih0 = inp["gru0_bih"].astype(f8); bhh0 = inp["gru0_bhh"].astype(f8)
    Wpre = Wz @ Wih0.T                  # [196, 1464]
    bpre = bz @ Wih0.T + bih0
    bpre[: 2 * REC] += bhh0[: 2 * REC]  # rz biases fully folded; n keeps bhh separate

    def chunk489(WT, bias_row, ncols):
        """[488, ncols] + bias row -> [4, 123, ncols]; row 122 of c<3 zero, c=3 bias."""
        out = np.zeros((4, KC + 1, ncols), np.float32)
        for c in range(4):
            out[c, :KC] = WT[c * KC:(c + 1) * KC].astype(np.float32)
        out[3, KC] = bias_row.astype(np.float32)
        return out

    z35 = np.zeros(2 * REC)
    # GRU0 h-stream: rhs = Whh0.T, bias row = [0(rz), bhh0_n]
    Whh0T = inp["gru0_Whh"].astype(f8).T
    wg0 = chunk489(Whh0T, np.concatenate([z35, bhh0[2 * REC:]]), G3)

    # GRU1 x-stream: rhs = Wih1.T, bias row = [bih1_rz + bhh1_rz, bih1_n]
    Wih1T = inp["gru1_Wih"].astype(f8).T
    bih1 = inp["gru1_bih"].astype(f8); bhh1 = inp["gru1_bhh"].astype(f8)
    bx1 = np.concatenate([bih1[: 2 * REC] + bhh1[: 2 * REC], bih1[2 * REC:]])
    wg1x = chunk489(Wih1T, bx1, G3)

    # GRU1 h-stream: rhs = Whh1.T, bias row = [0, bhh1_n]
    Whh1T = inp["gru1_Whh"].astype(f8).T
    wg1h = chunk489(Whh1T, np.concatenate([z35, bhh1[2 * REC:]]), G3)

    # cell x-stream: rhs = cell_Wih.T [488, 105], bias row = cell_bih
    wcx = chunk489(inp["cell_Wih"].astype(f8).T, inp["cell_bih"].astype(f8), 3 * NCH)

    wch = np.ascontiguousarray(inp["cell_Whh"][: 2 * NCH].T).astype(np.float32)  # [35,70]
    wcn = np.ascontiguousarray(inp["cell_Whh"][2 * NCH:].T).astype(np.float32)   # [35,35]

    # prefix matmul: [2, 99, 1464]; K rows 0:196 = Wpre, row 196 = bpre
    wpre = np.zeros((2, 99, G3), np.float32)
    Wpre_aug = np.concatenate([Wpre, bpre[None, :]], axis=0)  # [197, 1464]
    wpre[0] = Wpre_aug[0:99].astype(np.float32)
    wpre[1, 0:98] = Wpre_aug[99:197].astype(np.float32)

    identf = np.eye(64, dtype=np.float32)
    ones = np.ones((1, BC), np.float32)
    return dict(wpre=wpre, wg0=wg0, wg1x=wg1x, wg1h=wg1h, wcx=wcx,
                wch=wch, wcn=wcn, identf=identf, identr=identf, ones=ones)


def _prep_z(z_in):
    """Per-core z shard -> [2, 99, BC] feature-major with ones row at K=196."""
    shards = []
    for i in range(NCORES):
        zc = z_in[i * BC:(i + 1) * BC].astype(np.float32)   # [BC, 196]
        aug = np.concatenate([zc.T, np.ones((1, BC), np.float32)], axis=0)  # [197, BC]
        zf = np.zeros((2, 99, BC), np.float32)
        zf[0] = aug[0:99]
        zf[1, 0:98] = aug[99:197]
        shards.append(zf)
    return shards


def build_program(T, debug=False):
    """Build the Bass/Tile program for all 8 cores. Returns compiled nc."""
    import concourse.bacc as bacc
    import concourse.bass as bass
    import concourse.mybir as mybir
    import concourse.tile as tile
    from contextlib import ExitStack

    f32 = mybir.dt.float32
    f32r = mybir.dt.float32r
    AF = mybir.ActivationFunctionType
    AX = mybir.AxisListType

    nc = bacc.Bacc("TRN2", target_bir_lowering=False, debug=False,
                   num_devices=NCORES)

    d_z = nc.dram_tensor("z", [2, 99, BC], f32r, kind="ExternalInput").ap()
    d_wpre = nc.dram_tensor("wpre", [2, 99, G3], f32r, kind="ExternalInput").ap()
    d_wg0 = nc.dram_tensor("wg0", [4, 123, G3], f32r, kind="ExternalInput").ap()
    d_wg1x = nc.dram_tensor("wg1x", [4, 123, G3], f32r, kind="ExternalInput").ap()
    d_wg1h = nc.dram_tensor("wg1h", [4, 123, G3], f32r, kind="ExternalInput").ap()
    d_wcx = nc.dram_tensor("wcx", [4, 123, 3 * NCH], f32, kind="ExternalInput").ap()
    d_wch = nc.dram_tensor("wch", [NCH, 2 * NCH], f32, kind="ExternalInput").ap()
    d_wcn = nc.dram_tensor("wcn", [NCH, NCH], f32, kind="ExternalInput").ap()
    d_identf = nc.dram_tensor("identf", [64, 64], f32, kind="ExternalInput").ap()
    d_identr = nc.dram_tensor("identr", [64, 64], f32r, kind="ExternalInput").ap()
    d_ones = nc.dram_tensor("ones", [1, BC], f32r, kind="ExternalInput").ap()
    d_out = nc.dram_tensor("out", [BC, T, NCH], f32, kind="ExternalOutput").ap()
    if debug:
        d_dbg_xg0 = nc.dram_tensor("dbg_xg0", [BC, G3], f32, kind="ExternalOutput").ap()
        d_dbg_h0 = nc.dram_tensor("dbg_h0", [BC, REC], f32, kind="ExternalOutput").ap()
        d_dbg_h1 = nc.dram_tensor("dbg_h1", [BC, REC], f32, kind="ExternalOutput").ap()
        d_dbg_rz0 = nc.dram_tensor("dbg_rz0", [BC, 1024], f32, kind="ExternalOutput").ap()
        d_dbg_rz1 = nc.dram_tensor("dbg_rz1", [BC, 1024], f32, kind="ExternalOutput").ap()
        d_dbg_h0T = nc.dram_tensor("dbg_h0T", [123, 4, BC], f32r, kind="ExternalOutput").ap()
        d_dbg_v = nc.dram_tensor("dbg_v", [BC, NCH], f32, kind="ExternalOutput").ap()
        d_dbg_ru = nc.dram_tensor("dbg_ru", [BC, 2 * NCH], f32, kind="ExternalOutput").ap()
        d_dbg_ns = nc.dram_tensor("dbg_ns", [BC, NCH], f32, kind="ExternalOutput").ap()

    with ExitStack() as ctx:
        tc = ctx.enter_context(tile.TileContext(nc))
        wp = ctx.enter_context(tc.tile_pool(name="wp", bufs=1))
        sp = ctx.enter_context(tc.tile_pool(name="sp", bufs=2))
        pp = ctx.enter_context(tc.tile_pool(name="pp", bufs=1, space="PSUM"))

        # ---- persistent SBUF ----
        w_pre = wp.tile([99, 2, G3], f32r, tag="w_pre")
        zfm = wp.tile([99, 2, BC], f32r, tag="zfm")
        w_g0 = wp.tile([123, 4, G3], f32r, tag="w_g0")
        w_g1x = wp.tile([123, 4, G3], f32r, tag="w_g1x")
        w_g1h = wp.tile([123, 4, G3], f32r, tag="w_g1h")
        w_cx = wp.tile([123, 4, 3 * NCH], f32, tag="w_cx")
        w_ch = wp.tile([NCH, 2 * NCH], f32, tag="w_ch")
        w_cn = wp.tile([NCH, NCH], f32, tag="w_cn")
        identf = wp.tile([64, 64], f32, tag="identf")
        identr = wp.tile([64, 64], f32r, tag="identr")
        xg0rz = wp.tile([BC, 1024], f32r, tag="xg0rz")  # bank-aligned rz (+pad)
        xg0n = wp.tile([BC, REC], f32, tag="xg0n")
        out_sb = wp.tile([BC, T, NCH], f32, tag="out_sb")
        h0T = wp.tile([123, 4, BC], f32r, tag="h0T")
        h1T = wp.tile([123, 4, BC], f32r, tag="h1T")
        hcT = wp.tile([NCH, BC], f32, tag="hcT")
        hc_zero = wp.tile([BC, NCH], f32, tag="hc_zero")
        scratch = wp.tile([BC, 2], f32, tag="scratch")

        for c in range(2):
            nc.sync.dma_start(w_pre[:, c, :], d_wpre[c])
            nc.sync.dma_start(zfm[:, c, :], d_z[c])
        for c in range(4):
            nc.sync.dma_start(w_g0[:, c, :], d_wg0[c])
            nc.sync.dma_start(w_g1x[:, c, :], d_wg1x[c])
            nc.sync.dma_start(w_g1h[:, c, :], d_wg1h[c])
            nc.sync.dma_start(w_cx[:, c, :], d_wcx[c])
        nc.sync.dma_start(w_ch[:], d_wch[:])
        nc.sync.dma_start(w_cn[:], d_wcn[:])
        nc.sync.dma_start(identf[:], d_identf[:])
        nc.sync.dma_start(identr[:], d_identr[:])

        nc.gpsimd.memset(h0T[:].bitcast(f32), 0.0)
        nc.gpsimd.memset(h1T[:].bitcast(f32), 0.0)
        nc.gpsimd.memset(hcT[:], 0.0)
        nc.gpsimd.memset(hc_zero[:], 0.0)
        nc.gpsimd.memset(scratch[:], 0.0)
        nc.vector.memset(xg0rz[:, 976:1024].bitcast(f32), 0.0)
        nc.sync.dma_start(h0T[122:123, 3, :], d_ones[:])
        nc.sync.dma_start(h1T[122:123, 3, :], d_ones[:])

        # ---- PSUM (exactly 8 banks) ----
        pg0 = pp.tile([BC, 3, 512], f32, tag="pg0")     # GRU0 gates
        pg1 = pp.tile([BC, 3, 512], f32, tag="pg1")     # GRU1 gates (bank2 = xn)
        phn1 = pp.tile([BC, 488], f32, tag="phn1")      # GRU1 h-side n gate
        pm_tr = pp.tile([123, 512], f32, tag="pm_tr")   # transposes + cell regions
        pm_cell = pm_tr

        TR = [slice(c * 64, (c + 1) * 64) for c in range(4)]  # transpose slots
        C_G = slice(256, 361)     # cell r,u,xn gates (one accumulation group)
        C_RU = slice(256, 326)
        C_XN = slice(326, 361)
        C_NM = slice(368, 403)
        C_TR = slice(408, 472)

        # warm the sigmoid_and_others ACT table before the loop
        nc.scalar.activation(scratch[:, 1:2], scratch[:, 0:1], AF.Sigmoid)

        # ---- prologue: xg0 = z2_aug @ Wpre_aug ----
        # rz -> pg0 banks 0,1 (tick 0's h-stream keeps accumulating there);
        # n  -> pg1 bank 2 (copied out before tick 1 clears it).
        for ci in range(2):
            st = ci == 0
            for (b, c0, w) in NSL:
                dst = pg0[:, b, 0:w] if b < 2 else pg1[:, 2, 0:w]
                nc.tensor.matmul(dst, zfm[0:99, ci, :], w_pre[0:99, ci, c0:c0 + w],
                                 start=st, stop=(ci == 1 and b == 2))
        nc.vector.tensor_copy(xg0rz[:, 0:512], pg0[:, 0, :])
        nc.vector.tensor_copy(xg0rz[:, 512:976], pg0[:, 1, 0:464])
        nc.vector.tensor_copy(xg0n[:], pg1[:, 2, 0:488])

        h0_prev = sp.tile([BC, REC], f32, tag="h0bm", name="h0_init")
        h1_prev = sp.tile([BC, REC], f32, tag="h1bm", name="h1_init")
        nc.gpsimd.memset(h0_prev[:], 0.0)
        nc.gpsimd.memset(h1_prev[:], 0.0)

        dbg = {}

        def gru_chain(layer, pg, hn_psum, xn_psum, xn_sbuf, h_prev):
            """Gate math + new state (batch-major) for one GRU layer."""
            rz = sp.tile([BC, 1024], f32, tag=f"rz{layer}", name=f"rz{layer}")
            dbg[f"rz{layer}"] = rz
            pgf = pg[:, :, :].rearrange("p a b -> p (a b)")
            # r first (it gates the critical path), z right after
            nc.scalar.activation(rz[:, 0:488], pgf[:, 0:488], AF.Sigmoid)
            nc.scalar.activation(rz[:, 488:976], pgf[:, 488:976], AF.Sigmoid)
            t2 = sp.tile([BC, REC], f32, tag=f"t2_{layer}", name=f"t2_{layer}")
            nc.vector.tensor_mul(t2[:], rz[:, 0:488], hn_psum)
            t3 = sp.tile([BC, REC], f32, tag=f"t3_{layer}", name=f"t3_{layer}")
            nc.vector.tensor_add(t3[:], t2[:],
                                 xn_psum if xn_psum is not None else xn_sbuf)
            n = sp.tile([BC, REC], f32, tag=f"n{layer}", name=f"n{layer}")
            nc.scalar.activation(n[:], t3[:], AF.Tanh)
            # off-critical: a = z*h_prev, w = 1-z  (ready before tanh finishes)
            a = sp.tile([BC, REC], f32, tag=f"a{layer}", name=f"a{layer}")
            nc.gpsimd.tensor_mul(a[:], rz[:, 488:976], h_prev[:])
            w_ = sp.tile([BC, REC], f32, tag=f"w{layer}", name=f"w{layer}")
            nc.vector.tensor_scalar(w_[:], rz[:, 488:976], -1.0, 1.0,
                                    mybir.AluOpType.mult, mybir.AluOpType.add)
            # critical tail: b = n*(1-z); h_new = a + b
            b = sp.tile([BC, REC], f32, tag=f"b{layer}", name=f"b{layer}")
            nc.vector.tensor_mul(b[:], n[:], w_[:])
            h_new = sp.tile([BC, REC], f32, tag=f"h{layer}bm", name=f"hnew{layer}")
            nc.vector.tensor_add(h_new[:], a[:], b[:])
            return h_new

        def transpose_state(h_new, hT):
            for c in range(4):
                nc.tensor.transpose(pm_tr[0:122, TR[c]],
                                    h_new[:, c * KC:(c + 1) * KC], identf[:])
            nc.vector.tensor_copy(
                hT[0:122, :, :].rearrange("p c n -> p (c n)"),
                pm_tr[0:122, 0:256])

        # ---- main loop ----
        for t in range(T + 2):
            do_g0 = t < T
            do_g1 = 1 <= t <= T
            do_cell = 2 <= t

            # --- PE streams (emit first: sets scheduler priority) ---
            if do_g0:
                if t > 0:
                    nc.tensor.matmul(pg0[:, 0, :], identr[:], xg0rz[:, 0:512],
                                     start=True, stop=False)
                    nc.tensor.matmul(pg0[:, 1, 0:464], identr[:], xg0rz[:, 512:976],
                                     start=True, stop=False)
                for c in range(4):
                    kk = 123 if c == 3 else 122
                    lhsT = h0T[0:kk, c, :]
                    last = c == 3
                    nc.tensor.matmul(pg0[:, 0, :], lhsT, w_g0[0:kk, c, 0:512],
                                     start=False, stop=last)
                    nc.tensor.matmul(pg0[:, 1, 0:464], lhsT, w_g0[0:kk, c, 512:976],
                                     start=False, stop=last)
                for c in range(4):
                    kk = 123 if c == 3 else 122
                    lhsT = h0T[0:kk, c, :]
                    nc.tensor.matmul(pg0[:, 2, 0:488], lhsT, w_g0[0:kk, c, 976:1464],
                                     start=(c == 0), stop=(c == 3))
            if do_g1:
                for c in range(4):
                    kk = 123 if c == 3 else 122
                    lhsT = h0T[0:kk, c, :]
                    nc.tensor.matmul(pg1[:, 0, :], lhsT, w_g1x[0:kk, c, 0:512],
                                     start=(c == 0), stop=False)
                    nc.tensor.matmul(pg1[:, 1, 0:464], lhsT, w_g1x[0:kk, c, 512:976],
                                     start=(c == 0), stop=False)
                for c in range(4):
                    kk = 123 if c == 3 else 122
                    lhsT = h1T[0:kk, c, :]
                    last = c == 3
                    nc.tensor.matmul(pg1[:, 0, :], lhsT, w_g1h[0:kk, c, 0:512],
                                     start=False, stop=last)
                    nc.tensor.matmul(pg1[:, 1, 0:464], lhsT, w_g1h[0:kk, c, 512:976],
                                     start=False, stop=last)
                for c in range(4):
                    kk = 123 if c == 3 else 122
                    nc.tensor.matmul(pg1[:, 2, 0:488], h0T[0:kk, c, :],
                                     w_g1x[0:kk, c, 976:1464],
                                     start=(c == 0), stop=(c == 3))
                for c in range(4):
                    kk = 123 if c == 3 else 122
                    nc.tensor.matmul(phn1[:], h1T[0:kk, c, :],
                                     w_g1h[0:kk, c, 976:1464],
                                     start=(c == 0), stop=(c == 3))
            if do_cell:
                # single accumulation group in the shared bank: exactly one
                # start=True opener (start clears has_written for the WHOLE
                # bank, so a second opener would wipe the group's bits)
                for c in range(4):
                    kk = 123 if c == 3 else 122
                    nc.tensor.matmul(pm_cell[0:64, C_G],
                                     h1T[0:kk, c, :].bitcast(f32),
                                     w_cx[0:kk, c, :],
                                     start=(c == 0), stop=False,
                                     skip_group_check=True)
                nc.tensor.matmul(pm_cell[0:64, C_RU], hcT[:], w_ch[:],
                                 start=False, stop=True, skip_group_check=True)

            # --- GRU0 elementwise + transpose ---
            if do_g0:
                h0_new = gru_chain(0, pg0, pg0[:, 2, 0:488], None,
                                   xg0n[:], h0_prev)
                if debug and t == 0:
                    dbg["rz0_first"] = dbg["rz0"]
                    dbg["h0_first"] = h0_new
                transpose_state(h0_new, h0T)
                h0_prev = h0_new

            # --- GRU1 elementwise + transpose ---
            if do_g1:
                h1_new = gru_chain(1, pg1, phn1[:], pg1[:, 2, 0:488], None, h1_prev)
                if debug and t == 1:
                    dbg["rz1_first"] = dbg["rz1"]
                    dbg["h1_first"] = h1_new
                transpose_state(h1_new, h1T)
                h1_prev = h1_new

            # --- cell ---
            if do_cell:
                s = t - 2
                hc_ap = hc_zero[:] if s == 0 else out_sb[:, s - 1, :]
                ru = sp.tile([BC, 2 * NCH], f32, tag="ru", name="ru")
                nc.scalar.activation(ru[:], pm_cell[0:64, C_RU], AF.Sigmoid)
                rh = sp.tile([BC, NCH], f32, tag="rh", name="rh")
                nc.vector.tensor_mul(rh[:], ru[:, 0:NCH], hc_ap)
                nc.tensor.transpose(pm_cell[0:NCH, C_TR], rh[:], identf[:])
                rhT = sp.tile([NCH, BC], f32, tag="rhT", name="rhT")
                nc.vector.tensor_copy(rhT[:], pm_cell[0:NCH, C_TR])
                nc.tensor.matmul(pm_cell[0:64, C_NM], rhT[:], w_cn[:],
                                 start=True, stop=True)
                xnc = sp.tile([BC, NCH], f32, tag="xnc", name="xnc")
                nc.vector.tensor_copy(xnc[:], pm_cell[0:64, C_XN])
                v = sp.tile([BC, NCH], f32, tag="v", name="v")
                nc.vector.tensor_add(v[:], xnc[:], pm_cell[0:64, C_NM])
                # softmax over the 35 features via sigmoid-exp trick
                nmax = sp.tile([BC, 2], f32, tag="nmax", name="nmax")
                nc.vector.reduce_max(nmax[:, 0:1], v[:], axis=AX.X, negate=True)
                nc.vector.tensor_scalar_mul(nmax[:, 1:2], nmax[:, 0:1], -1.0)
                pe_ = sp.tile([BC, NCH], f32, tag="pe_", name="pe_")
                nc.scalar.activation(pe_[:], v[:], AF.Sigmoid, bias=nmax[:, 0:1])
                qe = sp.tile([BC, NCH], f32, tag="qe", name="qe")
                nc.scalar.activation(qe[:], v[:], AF.Sigmoid, bias=nmax[:, 1:2],
                                     scale=-1.0)
                qr = sp.tile([BC, NCH], f32, tag="qr", name="qr")
                nc.vector.reciprocal(qr[:], qe[:])
                ex = sp.tile([BC, NCH], f32, tag="ex", name="ex")
                nc.vector.tensor_mul(ex[:], pe_[:], qr[:])
                ssum = sp.tile([BC, 2], f32, tag="ssum", name="ssum")
                nc.vector.reduce_sum(ssum[:, 0:1], ex[:], axis=AX.X)
                nc.vector.reciprocal(ssum[:, 1:2], ssum[:, 0:1])
                nsoft = sp.tile([BC, NCH], f32, tag="nsoft", name="nsoft")
                nc.vector.tensor_scalar_mul(nsoft[:], ex[:], ssum[:, 1:2])
                # hc2 = nsoft + u * (hc - nsoft)
                dc = sp.tile([BC, NCH], f32, tag="dc", name="dc")
                nc.gpsimd.tensor_sub(dc[:], hc_ap, nsoft[:])
                ec = sp.tile([BC, NCH], f32, tag="ec", name="ec")
                nc.vector.tensor_mul(ec[:], ru[:, NCH:2 * NCH], dc[:])
                nc.gpsimd.tensor_add(out_sb[:, s, :], nsoft[:], ec[:])
                # transpose new cell state for next tick
                nc.tensor.transpose(pm_cell[0:NCH, C_TR], out_sb[:, s, :], identf[:])
                nc.vector.tensor_copy(hcT[:], pm_cell[0:NCH, C_TR])
                if debug and t == 2:
                    dbg["v"] = v; dbg["ru"] = ru; dbg["ns"] = nsoft

        if debug:
            nc.sync.dma_start(d_dbg_xg0[:, 0:976], xg0rz[:, 0:976].bitcast(f32))
            nc.sync.dma_start(d_dbg_xg0[:, 976:1464], xg0n[:])
            nc.sync.dma_start(d_dbg_h0, dbg["h0_first"][:])
            nc.sync.dma_start(d_dbg_h1, dbg["h1_first"][:])
            nc.sync.dma_start(d_dbg_rz0, dbg["rz0_first"][:])
            nc.sync.dma_start(d_dbg_rz1, dbg["rz1_first"][:])
            nc.sync.dma_start(d_dbg_h0T, h0T[:])
            nc.sync.dma_start(d_dbg_v, dbg["v"][:])
            nc.sync.dma_start(d_dbg_ru, dbg["ru"][:])
            nc.sync.dma_start(d_dbg_ns, dbg["ns"][:])
        nc.sync.dma_start(d_out, out_sb[:])

    nc.compile()
    return nc


def run(inputs, T=T_FULL, trace=False):
    from concourse.bass_utils import run_bass_kernel_spmd

    w = _prep_weights(inputs)
    zs = _prep_z(np.asarray(inputs["z_in"]))
    nc = build_program(T)
    in_maps = [dict(w, z=zs[i]) for i in range(NCORES)]
    res = run_bass_kernel_spmd(nc, in_maps, list(range(NCORES)), trace=trace)
    out = np.concatenate([res.results[i]["out"] for i in range(NCORES)], axis=0)
    return out.astype(np.float32), res


def kernel(**inputs):
    out, _ = run(inputs, T=T_FULL, trace=False)
    return out

